# revision 1
# baseline (speedup 1.0000x reference)
"""Multi-head causal self-attention (B=64, T=256, C=384, H=6) on 8 NeuronCores.

Data-parallel over batch: each core processes 8 batches (2048 tokens).
All on-device tensors are laid out so no device-side transposes are needed:
  - xT, Q.T, K.T feature-major [C, tokens]
  - V token-major [tokens, C]
  - scores computed transposed (S.T[tk, tq]) so exp(S.T) feeds P.T@V directly
  - attention output lands feature-major (catT) for the output projection
Matmul operands are bf16 (fp32 matmul is two-pass on trn2); accumulation,
softmax denominators, normalization and the final output stay fp32.
Heads are processed in pairs: score matmuls row-pack (K=64 at partition 0/64),
O.T matmuls col-pack (M=64 at array columns 0/64) into one [128,256] PSUM
tile, so softmax normalization is one tensor_tensor per pair.
"""

import sys

import ml_dtypes
import numpy as np

for _p in ("/opt/trn_rl_repo", "/root/.axon_site/_ro/trn_rl_repo"):
    if _p not in sys.path:
        sys.path.insert(0, _p)

import concourse.bass as bass
import concourse.tile as tile
from concourse import bacc, mybir
from concourse.bass_utils import run_bass_kernel_spmd

B, T, C, H, D = 64, 256, 384, 6, 64
NCORES = 8
BB = B // NCORES  # batches per core = 8
TOK = BB * T      # tokens per core = 2048
SCALE = float(C) ** -0.5
F32 = mybir.dt.float32
BF16 = mybir.dt.bfloat16
NPBF = ml_dtypes.bfloat16

NT4 = TOK // 512  # 4 column-chunks of 512 tokens
NKC = C // 128    # 3 chunks of 128 over feature dim


def build_module():
    nc = bacc.Bacc("TRN2", target_bir_lowering=False, debug=False)

    xT = nc.dram_tensor("xT", [C, TOK], BF16, kind="ExternalInput").ap()
    wall = nc.dram_tensor("wall", [C, 4 * C], BF16, kind="ExternalInput").ap()
    wobc = nc.dram_tensor("wobc", [C, 1], F32, kind="ExternalInput").ap()
    ones = nc.dram_tensor("ones", [128, 32], BF16, kind="ExternalInput").ap()
    yT = nc.dram_tensor("yT", [C, TOK], F32, kind="ExternalOutput").ap()
    # DRAM scratch for the per-batch reciprocal rows (enables broadcast DMA)
    zdram = nc.dram_tensor("zdram", [BB, H * 256], F32).ap()

    with tile.TileContext(nc) as tc:
        import contextlib

        ctx = contextlib.ExitStack()
        with ctx:
            consts = ctx.enter_context(tc.tile_pool(name="consts", bufs=1))

            # ---- persistent SBUF tiles ----
            def ptile(name, shape, dt=BF16):
                return consts.tile(shape, dt, tag=name, name=name)

            wall_sb = [ptile(f"wall{k}", [128, 4 * C]) for k in range(NKC)]
            wq_sb = [w[:, 0:C] for w in wall_sb]
            wk_sb = [w[:, C:2 * C] for w in wall_sb]
            wv_sb = [w[:, 2 * C:3 * C] for w in wall_sb]
            wo_sb = [w[:, 3 * C:4 * C] for w in wall_sb]
            wob_sb = [ptile(f"wob{k}", [128, 1], F32) for k in range(NKC)]
            ones_sb = ptile("ones", [128, 32])
            xt_sb = [ptile(f"xt{k}", [128, TOK]) for k in range(NKC)]
            qt_sb = [[ptile(f"qt{k}_{t}", [128, 512]) for t in range(NT4)] for k in range(NKC)]
            kt_sb = [[ptile(f"kt{k}_{t}", [128, 512]) for t in range(NT4)] for k in range(NKC)]
            cat_sb = [[ptile(f"cat{k}_{t}", [128, 512]) for t in range(NT4)] for k in range(NKC)]
            v_sb = [ptile(f"v{t}", [128, C]) for t in range(2 * BB)]  # 16 token-blocks of 128

            # ---- input DMAs (weights on gpsimd, x halves split sync/scalar) ----
            for k in range(NKC):
                nc.gpsimd.dma_start(out=wall_sb[k], in_=wall[128 * k:128 * (k + 1), :])
            for k in range(NKC):
                nc.sync.dma_start(
                    out=xt_sb[k][:, 0:TOK // 2],
                    in_=xT[128 * k:128 * (k + 1), 0:TOK // 2],
                )
                nc.scalar.dma_start(
                    out=xt_sb[k][:, TOK // 2:TOK],
                    in_=xT[128 * k:128 * (k + 1), TOK // 2:TOK],
                )
            nc.gpsimd.dma_start(out=ones_sb, in_=ones)
            for k in range(NKC):
                nc.gpsimd.dma_start(out=wob_sb[k], in_=wobc[128 * k:128 * (k + 1), :])

            # ---- PSUM pools (8 banks: pa 2 + ps 2x2 + po 2) ----
            pa = ctx.enter_context(tc.tile_pool(name="pa", bufs=2, space="PSUM"))
            ps = ctx.enter_context(tc.tile_pool(name="ps", bufs=2, space="PSUM"))
            po = ctx.enter_context(tc.tile_pool(name="po", bufs=2, space="PSUM"))

            # working SBUF pools (PT: one tile per (b,pair) -> no slot-reuse waits)
            pt_pool = ctx.enter_context(tc.tile_pool(name="ptp", bufs=BB * H // 2))
            rp_pool = ctx.enter_context(tc.tile_pool(name="rpp", bufs=3))
            bc_pool = ctx.enter_context(tc.tile_pool(name="bcp", bufs=3))
            y_pool = ctx.enter_context(tc.tile_pool(name="yp", bufs=3))

            # ---- phase 1a: Q.T / K.T = W @ x.T, feature-major [C, tok] ----
            for t in range(NT4):
                for which, w_sb, out_sb in (("q", wq_sb, qt_sb), ("k", wk_sb, kt_sb)):
                    for co in range(NKC):
                        pqk = pa.tile([128, 512], F32, tag="pa", name=f"p{which}{co}_{t}")
                        for kc in range(NKC):
                            nc.tensor.matmul(
                                pqk,
                                w_sb[kc][:, 128 * co:128 * (co + 1)],
                                xt_sb[kc][:, 512 * t:512 * (t + 1)],
                                start=(kc == 0),
                                stop=(kc == NKC - 1),
                            )
                        if which == "q":
                            nc.scalar.copy(out_sb[co][t], pqk)
                        else:
                            nc.vector.tensor_copy(out_sb[co][t], pqk)

            # ---- phase 1b: V token-major [tok, C] ----
            for tb in range(2 * BB):
                pv = pa.tile([128, C], F32, tag="pa", name=f"pv{tb}")
                for kc in range(NKC):
                    nc.tensor.matmul(
                        pv,
                        xt_sb[kc][:, 128 * tb:128 * (tb + 1)],
                        wv_sb[kc],
                        start=(kc == 0),
                        stop=(kc == NKC - 1),
                    )
                nc.scalar.copy(v_sb[tb], pv)

            # ---- phase 2: attention, head pairs ----
            for b in range(BB):
                t4b, qc = b // 2, (b % 2) * 256  # 512-tile index / col offset for this batch
                # Z rows land at psum partitions {0,32,64}: one [65,256] tile per
                # 3 heads, M=1 ones-column matmuls col-packed at 3 positions
                pzz = [pa.tile([96, 256], F32, tag="pa", name=f"pzz{b}_{g}")
                       for g in range(2)]
                po_tiles = []
                for hp in range(H // 2):
                    # scores for the pair: h0 at cols 0:384 (bank 0), h1 at 512:896
                    # (bank 1); within a head: cols +0:256 = tk-blk0 x tq 0:256,
                    # cols +256:384 = tk-blk1 x tq 128:256
                    p_s = ps.tile([128, 1024], F32, tag="ps", name=f"s{b}_{hp}")
                    for hh in range(2):
                        h = 2 * hp + hh
                        r0, s0 = 64 * hh, 512 * hh
                        qt = qt_sb[hp][t4b]
                        kt = kt_sb[hp][t4b]
                        nc.tensor.matmul(
                            p_s[:, s0:s0 + 256],
                            kt[r0:r0 + 64, qc:qc + 128],
                            qt[r0:r0 + 64, qc:qc + 256],
                            start=True, stop=True,
                        )
                        nc.tensor.matmul(
                            p_s[:, s0 + 256:s0 + 384],
                            kt[r0:r0 + 64, qc + 128:qc + 256],
                            qt[r0:r0 + 64, qc + 128:qc + 256],
                            start=True, stop=True,
                        )
                    # P.T = exp(S.T / sqrt(C)) for both heads in one op
                    pt = pt_pool.tile([128, 768], BF16, tag="pt", name=f"pt{b}_{hp}")
                    nc.scalar.activation(
                        pt.rearrange("p (a q) -> p a q", q=384),
                        p_s.rearrange("p (a q) -> p a q", q=512)[:, :, 0:384],
                        mybir.ActivationFunctionType.Exp, scale=SCALE,
                    )
                    # causal mask on the diagonal blocks (ISA allows 2 free dims)
                    for hh in range(2):
                        sel = pt[:, 384 * hh:384 * (hh + 1)] \
                            .rearrange("p (c i) -> p c i", i=128)[:, 0::2, :]
                        nc.gpsimd.affine_select(
                            out=sel, in_=sel,
                            pattern=[[0, 2], [1, 128]],
                            compare_op=mybir.AluOpType.is_ge,
                            fill=0.0, base=0, channel_multiplier=-1,
                        )
                    # O.T for the pair: h0 -> psum rows 0:64 (array cols 0:64),
                    # h1 -> psum rows 64:128 (array cols 64:128)
                    p_o = po.tile([128, 256], F32, tag="po", name=f"po{b}_{hp}")
                    po_tiles.append(p_o)
                    for tkb in range(2):
                        for hh in range(2):
                            h = 2 * hp + hh
                            r0, s0 = 64 * hh, 512 * hh
                            if tkb == 0:
                                nc.tensor.matmul(
                                    p_o[r0:r0 + 64, 0:256],
                                    v_sb[2 * b][:, 64 * h:64 * (h + 1)],
                                    pt[:, 384 * hh:384 * hh + 256],
                                    start=True, stop=False,
                                    tile_position=(0, r0),
                                    skip_group_check=True,
                                )
                            else:
                                nc.tensor.matmul(
                                    p_o[r0:r0 + 64, 128:256],
                                    v_sb[2 * b + 1][:, 64 * h:64 * (h + 1)],
                                    pt[:, 384 * hh + 256:384 * hh + 384],
                                    start=False, stop=True,
                                    tile_position=(0, r0),
                                    skip_group_check=True,
                                )
                    # Z-gather: softmax denominators via M=1 ones-column matmuls,
                    # head h -> row 32*(h%3) of pzz[h//3]
                    for hh in range(2):
                        h = 2 * hp + hh
                        g, zr = h // 3, 32 * (h % 3)
                        nc.tensor.matmul(
                            pzz[g][zr:zr + 32, 0:256],
                            ones_sb,
                            pt[:, 384 * hh:384 * hh + 256],
                            start=True, stop=False,
                            tile_position=(0, zr),
                            skip_group_check=True,
                        )
                        nc.tensor.matmul(
                            pzz[g][zr:zr + 32, 128:256],
                            ones_sb,
                            pt[:, 384 * hh + 256:384 * hh + 384],
                            start=False, stop=True,
                            tile_position=(0, zr),
                            skip_group_check=True,
                        )
                # denominators -> reciprocals (rows {0,32,64} are the 3 heads of
                # each group; the rows in between are unused)
                for g in range(2):
                    rp = rp_pool.tile([96, 256], F32, tag="rp", name=f"rp{b}_{g}")
                    nc.vector.reciprocal_approx_fast(rp, pzz[g])
                    rps = rp[0:96:32, :]
                    nc.sync.dma_start(
                        out=zdram[b:b + 1, 768 * g:768 * (g + 1)]
                            .rearrange("o (h q) -> (o h) q", q=256),
                        in_=rps,
                    )
                bc = bc_pool.tile([128, (H // 2) * 256], F32, tag="bc", name=f"bc{b}")
                zsrc = zdram[b, :]
                for hh in range(2):
                    bc_src = bass.AP(
                        tensor=zsrc.tensor, offset=zsrc.offset + 256 * hh,
                        ap=[[0, 64], [512, H // 2], [1, 256]],
                    )
                    nc.sync.dma_start(out=bc[64 * hh:64 * (hh + 1), :], in_=bc_src)
                # normalize each pair's O.T into catT (one op per pair)
                for hp in range(H // 2):
                    nc.vector.tensor_mul(
                        cat_sb[hp][t4b][:, qc:qc + 256],
                        po_tiles[hp],
                        bc[:, 256 * hp:256 * (hp + 1)],
                    )

            # ---- phase 3: y.T = Wo @ catT + bo ----
            for t in range(NT4):
                for co in range(NKC):
                    pyk = pa.tile([128, 512], F32, tag="pa", name=f"py{co}_{t}")
                    for kc in range(NKC):
                        nc.tensor.matmul(
                            pyk,
                            wo_sb[kc][:, 128 * co:128 * (co + 1)],
                            cat_sb[kc][t],
                            start=(kc == 0),
                            stop=(kc == NKC - 1),
                        )
                    yt = y_pool.tile([128, 512], F32, tag="yt", name=f"yt{co}_{t}")
                    nc.vector.tensor_scalar_add(yt, pyk, wob_sb[co][:, 0:1])
                    nc.sync.dma_start(
                        out=yT[128 * co:128 * (co + 1), 512 * t:512 * (t + 1)],
                        in_=yt,
                    )

    nc.compile()
    return nc


def make_in_maps(x, Wk, Wq, Wv, Wo, bo):
    x = np.asarray(x, np.float32)
    wall = np.concatenate(
        [np.asarray(w, np.float32).T for w in (Wq, Wk, Wv, Wo)], axis=1
    ).astype(NPBF)
    wobc = np.ascontiguousarray(np.asarray(bo, np.float32).reshape(C, 1))
    ones = np.ones((128, 32), NPBF)
    in_maps = []
    for i in range(NCORES):
        xi = x[BB * i:BB * (i + 1)].reshape(TOK, C)
        in_maps.append({
            "xT": np.ascontiguousarray(xi.T).astype(NPBF),
            "wall": wall, "wobc": wobc, "ones": ones,
        })
    return in_maps


_NC_CACHE = None


def kernel(x, Wk, Wq, Wv, Wo, bo):
    global _NC_CACHE
    if _NC_CACHE is None:
        _NC_CACHE = build_module()
    nc = _NC_CACHE
    in_maps = make_in_maps(x, Wk, Wq, Wv, Wo, bo)
    res = run_bass_kernel_spmd(nc, in_maps, core_ids=list(range(NCORES)))
    outs = []
    for i in range(NCORES):
        yt = np.asarray(res.results[i]["yT"])
        outs.append(yt.T.reshape(BB, T, C))
    return np.concatenate(outs, axis=0).astype(np.float32)

